# revision 11
# baseline (speedup 1.0000x reference)
"""Trainium2 Bass kernel for nn_MinibatchDiscriminator (N=512, INSIZE=512, K=64, D=16).

Per core (row-shard of 64 i's, full j range), fp16 pipeline:
  feat = x @ W.T computed as featH chunks [128=(8k x 16d), 512 j] fp16
  (bias b cancels in all pairwise differences and is dropped).
  Per group of 2 i's: 16 elementwise absdiff-ish ops (13 DVE min, 3 ACT abs),
  d-reduction via 64x32-tiled PE matmuls into 2 PSUM banks per pair of
  groups (row-tile h -> bank h), B-correction + diagonal eraser as tiled
  matmuls in the same mode, then one ACT exp+accum per bank.
  o_b rows gathered via two permutation matmuls at the end.
"""
import sys

import numpy as np

sys.path.insert(0, "/opt/trn_rl_repo")

import concourse.bass as bass
import concourse.tile as tile
from concourse import bacc, mybir
from concourse.bass_utils import run_bass_kernel_spmd

AF = mybir.ActivationFunctionType
OP = mybir.AluOpType
FP32 = mybir.dt.float32
FP16 = mybir.dt.float16

N, INSIZE, K, D = 512, 512, 64, 16
KD = K * D
NCORES = 8
NL = N // NCORES          # 64 rows per core
P = 128
CH = KD // P              # 8 chunks of (8k x 16d)
NT = INSIZE // P          # 4 contraction tiles
NG = NL // 2              # 32 groups of 2 rows
NPAIR = NG // 2           # 16 pairs of groups (2 banks each)
ACT_SC = {(0, 3), (1, 3), (0, 7)}   # (s, c) absdiffs on ScalarE (abs rows)
BIG = 200.0

TRACE = False
_cache = {}


def _row_sc(row):
    """bank row (within [128]) -> (s, c, m) given col-group C=row//32."""
    C, r = row // 32, row % 32
    return C % 2, r // 4, r % 4   # s, c, m


def _build():
    nc = bacc.Bacc("TRN2", target_bir_lowering=False)
    xte_h = nc.dram_tensor("xte", [NT, P, 576], FP16, kind="ExternalInput").ap()
    wt_h = nc.dram_tensor("wt", [CH, P, NT, P], FP16, kind="ExternalInput").ap()
    w64_h = nc.dram_tensor("w64", [P, CH * 32], FP16, kind="ExternalInput").ap()
    iblk_h = nc.dram_tensor("iblk", [P, 64], FP16, kind="ExternalInput").ap()
    bigw_h = nc.dram_tensor("bigw", [P, 4 * 32], FP16, kind="ExternalInput").ap()
    ebuf_h = nc.dram_tensor("ebuf", [P, 1024], FP16, kind="ExternalInput").ap()
    scol_h = nc.dram_tensor("scol", [P, 1], FP32, kind="ExternalInput").ap()
    rmask_h = nc.dram_tensor("rmask", [P, 1], FP32, kind="ExternalInput").ap()
    nmask_h = nc.dram_tensor("nmask", [P, 1], FP32, kind="ExternalInput").ap()
    perm_h = nc.dram_tensor("perm", [2, P, 256], FP32, kind="ExternalInput").ap()
    xs_h = nc.dram_tensor("xs", [NL, INSIZE], FP32, kind="ExternalInput").ap()
    out_h = nc.dram_tensor("out", [NL, INSIZE + K], FP32, kind="ExternalOutput").ap()

    with tile.TileContext(nc) as tc:
        with (
            tc.tile_pool(name="cst", bufs=1) as cst,
            tc.tile_pool(name="inp", bufs=1) as inp,
            tc.tile_pool(name="ad", bufs=84) as adp,
            tc.tile_pool(name="scr", bufs=4) as scp,
            tc.tile_pool(name="fps", bufs=1, space="PSUM") as fps,
            tc.tile_pool(name="fp2", bufs=2, space="PSUM") as fp2,
            tc.tile_pool(name="nps", bufs=2, space="PSUM") as nps,
            tc.tile_pool(name="ops", bufs=1, space="PSUM") as ops,
        ):
            # ---------------- input DMAs ----------------
            xte_sb = []
            for t in range(NT):
                s = inp.tile([P, 576], FP16, tag=f"xte{t}", name=f"xte{t}")
                nc.sync.dma_start(out=s, in_=xte_h[t])
                xte_sb.append(s)
            wt_sb = []
            for c in range(CH):
                s = inp.tile([P, NT, P], FP16, tag=f"wt{c}", name=f"wt{c}")
                nc.scalar.dma_start(out=s, in_=wt_h[c])
                wt_sb.append(s)
            w64_sb = cst.tile([P, CH * 32], FP16, tag="w64")
            nc.gpsimd.dma_start(out=w64_sb, in_=w64_h)
            iblk_sb = cst.tile([P, 64], FP16, tag="iblk")
            nc.gpsimd.dma_start(out=iblk_sb, in_=iblk_h)
            bigw_sb = cst.tile([P, 4 * 32], FP16, tag="bigw")
            nc.gpsimd.dma_start(out=bigw_sb, in_=bigw_h)
            ebuf_sb = cst.tile([P, 1024], FP16, tag="ebuf")
            nc.gpsimd.dma_start(out=ebuf_sb, in_=ebuf_h)
            scol_sb = cst.tile([P, 1], FP32, tag="scol")
            nc.gpsimd.dma_start(out=scol_sb, in_=scol_h)
            rmask_sb = cst.tile([P, 1], FP32, tag="rmask")
            nc.gpsimd.dma_start(out=rmask_sb, in_=rmask_h)
            nmask_sb = cst.tile([P, 1], FP32, tag="nmask")
            nc.gpsimd.dma_start(out=nmask_sb, in_=nmask_h)
            perm_sb = cst.tile([P, 2, 256], FP32, tag="perm")
            nc.scalar.dma_start(out=perm_sb, in_=perm_h.rearrange("h p f -> p h f"))
            # x passthrough
            nc.gpsimd.dma_start(out=out_h[:, 0:INSIZE], in_=xs_h)

            # ---------------- featH / fCol per chunk ----------------
            featH, fCol, fC16 = [], [], []
            early_ads = {}
            for c in range(CH):
                psf = fp2.tile([P, 512], FP32, tag="psf", name="psf")
                for t in range(NT):
                    nc.tensor.matmul(
                        psf, wt_sb[c][:, t, :], xte_sb[t][:, 0:512],
                        start=(t == 0), stop=(t == NT - 1),
                    )
                psg = fps.tile([P, 64], FP32, tag="psg", name="psg")
                for t in range(NT):
                    nc.tensor.matmul(
                        psg, wt_sb[c][:, t, :], xte_sb[t][:, 512:576],
                        start=(t == 0), stop=(t == NT - 1),
                    )
                fh = inp.tile([P, 512], FP16, tag=f"fh{c}", name=f"fh{c}")
                (nc.vector.tensor_copy if c % 2 else nc.scalar.copy)(fh, psf)
                featH.append(fh)
                fc = inp.tile([P, 64], FP32, tag=f"fc{c}", name=f"fc{c}")
                nc.vector.tensor_copy(fc, psg)
                fCol.append(fc)
                f16 = inp.tile([P, 64], FP16, tag=f"f16_{c}", name=f"f16_{c}")
                nc.vector.tensor_copy(f16, psg)
                fC16.append(f16)
                # issue pairs 0-1 elementwise ops for this chunk right away
                for p in range(2):
                    for gam in range(2):
                        for s in range(2):
                            i = 2 * (2 * p + gam) + s
                            ad = adp.tile([P, 512], FP16, tag="ad", name="ad")
                            col = fc[:, i : i + 1]
                            if (s, c) in ACT_SC:
                                nc.scalar.activation(
                                    ad, fh, AF.Abs, bias=col, scale=-1.0
                                )
                            else:
                                nc.vector.tensor_scalar(
                                    ad, fh, col, None, op0=OP.min
                                )
                            early_ads[(p, gam, s, c)] = ad

            def make_ads(p):
                ads = {}
                for gam in range(2):
                    g = 2 * p + gam
                    for s in range(2):
                        i = 2 * g + s
                        for c in range(CH):
                            ad = adp.tile([P, 512], FP16, tag="ad", name="ad")
                            col = fCol[c][:, i : i + 1]
                            if (s, c) in ACT_SC:
                                nc.scalar.activation(
                                    ad, featH[c], AF.Abs, bias=col, scale=-1.0
                                )
                            else:
                                nc.vector.tensor_scalar(
                                    ad, featH[c], col, None, op0=OP.min
                                )
                            ads[(gam, s, c)] = ad
                return ads

            # ---------------- B and A tables (tiled reduce of featH/fCol) ----------------
            # B banks: one per h; rows 32C+4c+m = B[k=8c+4h+m, j] for every C.
            bbank = []
            for h in range(2):
                bb = nps.tile([P, 512], FP32, tag=f"nb{h}", name=f"bb{h}")
                for c in range(CH):
                    for C in range(4):
                        nc.tensor.matmul(
                            bb[32 * C : 32 * C + 32, :],
                            w64_sb[64 * h : 64 * h + 64, 32 * c : 32 * c + 32],
                            featH[c][64 * h : 64 * h + 64, :],
                            start=(c == 0), stop=(c == CH - 1),
                            tile_position=(64 * h, 32 * C),
                        )
                bbank.append(bb)
            # A bank: rows 32h + 4c + m = A[i, k=8c+4h+m] over free i (64).
            a32s = []
            for h in range(2):
                ab = fps.tile([P, 64], FP32, tag="psg", name=f"abank{h}")
                for c in range(CH):
                    nc.tensor.matmul(
                        ab[0:32, :],
                        w64_sb[64 * h : 64 * h + 64, 32 * c : 32 * c + 32],
                        fC16[c][64 * h : 64 * h + 64, :],
                        start=(c == 0), stop=(c == CH - 1),
                        tile_position=(64 * h, 0),
                    )
                a32 = cst.tile([32, 64], FP32, tag=f"a32_{h}", name=f"a32_{h}")
                nc.vector.tensor_copy(a32, ab[0:32, :])
                a32s.append(a32)
            # B_dup_h = bbank_h * rmask (-0.5 on min rows, 0 on abs rows), fp16;
            # then duplicated into both partition halves per C-pair (lo: C0|C1, hi: C2|C3)
            bdup = {}
            for h in range(2):
                bd = cst.tile([P, 512], FP16, tag=f"bd{h}", name=f"bd{h}")
                nc.vector.tensor_scalar(bd, bbank[h], rmask_sb[:, 0:1], None, op0=OP.mult)
                for half, nm in ((0, "lo"), (1, "hi")):
                    dd = cst.tile([P, 512], FP16, tag=f"bd{h}{nm}", name=f"bd{h}{nm}")
                    for a in range(2):
                        nc.gpsimd.dma_start(
                            out=dd[64 * a : 64 * a + 64, :],
                            in_=bd[64 * half : 64 * half + 64, :],
                        )
                    bdup[(h, half)] = dd
            # negA_h [128, NPAIR]: row 32C+4c+m, col p = -A[i=4p+C, k] (min rows)
            # replicate abank row block to all four col-group row ranges (SBUF copy via DMA)
            arep = []
            for h in range(2):
                at = cst.tile([P, 64], FP32, tag=f"arep{h}", name=f"arep{h}")
                for C in range(4):
                    nc.gpsimd.dma_start(out=at[32 * C : 32 * C + 32, :], in_=a32s[h])
                arep.append(at)
            negA = []
            for h in range(2):
                na = cst.tile([P, NPAIR], FP32, tag=f"na{h}", name=f"na{h}")
                for C in range(4):
                    nc.vector.tensor_copy(
                        na[32 * C : 32 * C + 32, :],
                        arep[h][32 * C : 32 * C + 32, C : C + 4 * (NPAIR - 1) + 1 : 4],
                    )
                nc.vector.tensor_scalar(na, na, nmask_sb[:, 0:1], None, op0=OP.mult)
                negA.append(na)

            o_raw = cst.tile([P, NG], FP32, tag="o_raw")

            # ---------------- phase 1 ----------------
            for p in range(NPAIR):
                if p < 2:
                    ads = {k[1:]: v for k, v in early_ads.items() if k[0] == p}
                else:
                    ads = make_ads(p)
                banks = []
                for h in range(2):
                    bk = nps.tile([P, 512], FP32, tag=f"nb{h}", name=f"nb{h}")
                    banks.append(bk)
                # d-reduce: c-major for weight reuse across the 8 slots
                for c in range(CH):
                    for gam in range(2):
                        for s in range(2):
                            C = 2 * gam + s
                            for h in range(2):
                                nc.tensor.matmul(
                                    banks[h][32 * C : 32 * C + 32, :],
                                    w64_sb[64 * h : 64 * h + 64, 32 * c : 32 * c + 32],
                                    ads[(gam, s, c)][64 * h : 64 * h + 64, :],
                                    start=(c == 0), stop=False,
                                    tile_position=(64 * h, 32 * C),
                                )
                # B correction (adds -0.5*B on min rows, 0 on abs rows)
                for h in range(2):
                    for C in range(4):
                        nc.tensor.matmul(
                            banks[h][32 * C : 32 * C + 32, :],
                            iblk_sb[64 * h : 64 * h + 64, 32 * (C % 2) : 32 * (C % 2) + 32],
                            bdup[(h, C // 2)][64 * h : 64 * h + 64, :],
                            start=False, stop=False,
                            tile_position=(64 * h, 32 * C),
                        )
                # eraser: -+BIG at the global diagonal column j = 64*core + 4p + C,
                # selected by the per-core indicator in ebuf (window offset is
                # compile-time, the core offset lives in the data).
                for h in range(2):
                    for C in range(4):
                        off = 512 - 4 * p - C
                        nc.tensor.matmul(
                            banks[h][32 * C : 32 * C + 32, :],
                            bigw_sb[64 * h : 64 * h + 64, 32 * C : 32 * C + 32],
                            ebuf_sb[64 * h : 64 * h + 64, off : off + 512],
                            start=False, stop=True,
                            tile_position=(64 * h, 32 * C),
                        )
                # exp + accumulate over j
                for h in range(2):
                    scr = scp.tile([P, 512], FP16, tag="scr", name="scr")
                    nc.scalar.activation(
                        scr, banks[h], AF.Exp,
                        bias=negA[h][:, p : p + 1],
                        scale=scol_sb[:, 0:1],
                        accum_out=o_raw[:, 2 * p + h : 2 * p + h + 1],
                    )

            # ---------------- epilogue ----------------
            out_ps = ops.tile([NPAIR, 256], FP32, tag="out_ps")
            for h in range(2):
                nc.tensor.matmul(
                    out_ps,
                    o_raw.rearrange("q (p h) -> q h p", h=2)[:, h, :],
                    perm_sb[:, h, :],
                    start=(h == 0), stop=(h == 1),
                )
            ob_sb = cst.tile([NPAIR, 256], FP32, tag="ob")
            nc.vector.tensor_copy(ob_sb, out_ps)
            out_ob = out_h[:, INSIZE : INSIZE + K].rearrange(
                "(p gs) k -> p gs k", gs=4
            )
            nc.sync.dma_start(
                out=out_ob, in_=ob_sb.rearrange("p (gs k) -> p gs k", gs=4)
            )

    nc.finalize()
    return nc


def _consts():
    # w64: pattern c maps contract row 16m+d -> out col 4c+m, both halves
    w64 = np.zeros((2, 64, CH, 32), np.float16)
    for a in range(2):
        for c in range(CH):
            for m in range(4):
                w64[a, 16 * m : 16 * m + 16, c, 4 * c + m] = 1.0
    w64 = np.ascontiguousarray(w64.reshape(P, CH * 32))
    # iblk: identity blocks for B correction
    iblk = np.zeros((2, 64, 64), np.float16)
    for a in range(2):
        for b in range(2):
            for r in range(32):
                iblk[a, 32 * b + r, 32 * b + r] = 1.0
    iblk = np.ascontiguousarray(iblk.reshape(P, 64))
    # min-row mask by bank row
    minrow = np.ones(P, bool)
    for row in range(P):
        s, c, m = _row_sc(row)
        if (s, c) in ACT_SC:
            minrow[row] = False
    # bigw: single contract-row weight, col value -BIG on min rows else +BIG
    bigw = np.zeros((2, 64, 4, 32), np.float16)
    for a in range(2):
        for C in range(4):
            for r in range(32):
                val = -BIG if minrow[32 * C + r] else BIG
                bigw[a, 0, C, r] = val
    bigw = np.ascontiguousarray(bigw.reshape(P, 4 * 32))

    scol = np.where(minrow, 2.0, -1.0).astype(np.float32).reshape(P, 1)
    rmask = np.where(minrow, -0.5, 0.0).astype(np.float32).reshape(P, 1)
    nmask = np.where(minrow, -1.0, 0.0).astype(np.float32).reshape(P, 1)
    # perm_h: row 32C+4c+m -> col C*64 + (8c+4h+m)
    perm = np.zeros((2, P, 256), np.float32)
    for h in range(2):
        for row in range(P):
            C = row // 32
            s, c, m = _row_sc(row)
            perm[h, row, C * 64 + 8 * c + 4 * h + m] = 1.0
    return w64, iblk, bigw, scol, rmask, nmask, perm


def kernel(x, W, b):
    x = np.asarray(x, np.float32)
    W = np.asarray(W, np.float32)
    if "nc" not in _cache:
        _cache["nc"] = _build()
    nc = _cache["nc"]
    w64, iblk, bigw, scol, rmask, nmask, perm = _consts()
    xT = np.ascontiguousarray(x.T)  # [INSIZE, N]
    # wt[c, p, t, kcol] = W[128c + kcol, 128t + p]
    wt = np.ascontiguousarray(
        W.reshape(CH, P, NT, P).transpose(0, 3, 2, 1).astype(np.float16)
    )
    in_maps = []
    for cidx in range(NCORES):
        xs = np.ascontiguousarray(x[NL * cidx : NL * (cidx + 1)])
        xte = np.concatenate([xT, xs.T], axis=1).astype(np.float16)  # [512, 576]
        xte_t = np.ascontiguousarray(xte.reshape(NT, P, 576))
        ebuf = np.zeros((P, 1024), np.float16)
        ebuf[0, 512 + NL * cidx] = 1.0
        ebuf[64, 512 + NL * cidx] = 1.0
        in_maps.append({
            "xte": xte_t, "wt": wt, "w64": w64, "iblk": iblk,
            "bigw": bigw, "ebuf": ebuf, "scol": scol, "rmask": rmask,
            "nmask": nmask, "perm": perm, "xs": xs,
        })
    res = run_bass_kernel_spmd(
        nc, in_maps, core_ids=list(range(NCORES)), trace=TRACE
    )
    _cache["last_results"] = res
    return np.ascontiguousarray(
        np.concatenate([res.results[c]["out"] for c in range(NCORES)], axis=0)
    )


# revision 13
# speedup vs baseline: 1.0219x; 1.0219x over previous
"""Trainium2 Bass kernel for nn_MinibatchDiscriminator (N=512, INSIZE=512, K=64, D=16).

Per core (row-shard of 64 i's, full j range), fp16 pipeline:
  feat = x @ W.T computed as featH chunks [128=(8k x 16d), 512 j] fp16
  (bias b cancels in all pairwise differences and is dropped).
  Per group of 2 i's: 16 elementwise absdiff-ish ops (13 DVE min, 3 ACT abs),
  d-reduction via 64x32-tiled PE matmuls into 2 PSUM banks per pair of
  groups (row-tile h -> bank h), B-correction + diagonal eraser as tiled
  matmuls in the same mode, then one ACT exp+accum per bank.
  o_b rows gathered via two permutation matmuls at the end.
"""
import sys

import numpy as np

sys.path.insert(0, "/opt/trn_rl_repo")

import concourse.bass as bass
import concourse.tile as tile
from concourse import bacc, mybir
from concourse.bass_utils import run_bass_kernel_spmd

AF = mybir.ActivationFunctionType
OP = mybir.AluOpType
FP32 = mybir.dt.float32
FP16 = mybir.dt.float16

N, INSIZE, K, D = 512, 512, 64, 16
KD = K * D
NCORES = 8
NL = N // NCORES          # 64 rows per core
P = 128
CH = KD // P              # 8 chunks of (8k x 16d)
NT = INSIZE // P          # 4 contraction tiles
NG = NL // 2              # 32 groups of 2 rows
NPAIR = NG // 2           # 16 pairs of groups (2 banks each)
ACT_SC = {(0, 3), (1, 3), (0, 7)}   # (s, c) absdiffs on ScalarE (abs rows)
BIG = 200.0

TRACE = False
_cache = {}


def _row_sc(row):
    """bank row (within [128]) -> (s, c, m) given col-group C=row//32."""
    C, r = row // 32, row % 32
    return C % 2, r // 4, r % 4   # s, c, m


def _build():
    nc = bacc.Bacc("TRN2", target_bir_lowering=False)
    xte_h = nc.dram_tensor("xte", [NT, P, 576], FP16, kind="ExternalInput").ap()
    wt_h = nc.dram_tensor("wt", [CH, P, NT, P], FP16, kind="ExternalInput").ap()
    w64_h = nc.dram_tensor("w64", [P, CH * 32], FP16, kind="ExternalInput").ap()
    iblk_h = nc.dram_tensor("iblk", [P, 64], FP16, kind="ExternalInput").ap()
    bigw_h = nc.dram_tensor("bigw", [P, 4 * 32], FP16, kind="ExternalInput").ap()
    ebuf_h = nc.dram_tensor("ebuf", [P, 1024], FP16, kind="ExternalInput").ap()
    scol_h = nc.dram_tensor("scol", [P, 1], FP32, kind="ExternalInput").ap()
    rmask_h = nc.dram_tensor("rmask", [P, 1], FP32, kind="ExternalInput").ap()
    nmask_h = nc.dram_tensor("nmask", [P, 1], FP32, kind="ExternalInput").ap()
    perm_h = nc.dram_tensor("perm", [2, P, 256], FP32, kind="ExternalInput").ap()
    xs_h = nc.dram_tensor("xs", [NL, INSIZE], FP32, kind="ExternalInput").ap()
    out_h = nc.dram_tensor("out", [NL, INSIZE + K], FP32, kind="ExternalOutput").ap()

    with tile.TileContext(nc) as tc:
        with (
            tc.tile_pool(name="cst", bufs=1) as cst,
            tc.tile_pool(name="inp", bufs=1) as inp,
            tc.tile_pool(name="ad", bufs=84) as adp,
            tc.tile_pool(name="scr", bufs=4) as scp,
            tc.tile_pool(name="fps", bufs=1, space="PSUM") as fps,
            tc.tile_pool(name="fp2", bufs=2, space="PSUM") as fp2,
            tc.tile_pool(name="nps", bufs=2, space="PSUM") as nps,
            tc.tile_pool(name="ops", bufs=1, space="PSUM") as ops,
        ):
            # ---------------- input DMAs ----------------
            xte_sb = []
            for t in range(NT):
                s = inp.tile([P, 576], FP16, tag=f"xte{t}", name=f"xte{t}")
                nc.sync.dma_start(out=s, in_=xte_h[t])
                xte_sb.append(s)
            wt_sb = []
            for c in range(CH):
                s = inp.tile([P, NT, P], FP16, tag=f"wt{c}", name=f"wt{c}")
                nc.scalar.dma_start(out=s, in_=wt_h[c])
                wt_sb.append(s)
            w64_sb = cst.tile([P, CH * 32], FP16, tag="w64")
            nc.gpsimd.dma_start(out=w64_sb, in_=w64_h)
            iblk_sb = cst.tile([P, 64], FP16, tag="iblk")
            nc.gpsimd.dma_start(out=iblk_sb, in_=iblk_h)
            bigw_sb = cst.tile([P, 4 * 32], FP16, tag="bigw")
            nc.gpsimd.dma_start(out=bigw_sb, in_=bigw_h)
            ebuf_sb = cst.tile([P, 1024], FP16, tag="ebuf")
            nc.gpsimd.dma_start(out=ebuf_sb, in_=ebuf_h)
            scol_sb = cst.tile([P, 1], FP32, tag="scol")
            nc.gpsimd.dma_start(out=scol_sb, in_=scol_h)
            rmask_sb = cst.tile([P, 1], FP32, tag="rmask")
            nc.gpsimd.dma_start(out=rmask_sb, in_=rmask_h)
            nmask_sb = cst.tile([P, 1], FP32, tag="nmask")
            nc.gpsimd.dma_start(out=nmask_sb, in_=nmask_h)
            perm_sb = cst.tile([P, 2, 256], FP32, tag="perm")
            nc.scalar.dma_start(out=perm_sb, in_=perm_h.rearrange("h p f -> p h f"))
            # x passthrough
            nc.gpsimd.dma_start(out=out_h[:, 0:INSIZE], in_=xs_h)

            # ---------------- featH / fCol per chunk ----------------
            featH, fCol, fC16 = [], [], []
            for c in range(CH):
                psf = fp2.tile([P, 512], FP32, tag="psf", name="psf")
                for t in range(NT):
                    nc.tensor.matmul(
                        psf, wt_sb[c][:, t, :], xte_sb[t][:, 0:512],
                        start=(t == 0), stop=(t == NT - 1),
                    )
                psg = fps.tile([P, 64], FP32, tag="psg", name="psg")
                for t in range(NT):
                    nc.tensor.matmul(
                        psg, wt_sb[c][:, t, :], xte_sb[t][:, 512:576],
                        start=(t == 0), stop=(t == NT - 1),
                    )
                fh = inp.tile([P, 512], FP16, tag=f"fh{c}", name=f"fh{c}")
                (nc.vector.tensor_copy if c % 2 else nc.scalar.copy)(fh, psf)
                featH.append(fh)
                fc = inp.tile([P, 64], FP32, tag=f"fc{c}", name=f"fc{c}")
                nc.vector.tensor_copy(fc, psg)
                fCol.append(fc)
                f16 = inp.tile([P, 64], FP16, tag=f"f16_{c}", name=f"f16_{c}")
                nc.vector.tensor_copy(f16, psg)
                fC16.append(f16)


            # ---------------- B and A tables (tiled reduce of featH/fCol) ----------------
            # B banks: one per h; rows 32C+4c+m = B[k=8c+4h+m, j] for every C.
            bbank = []
            for h in range(2):
                bb = nps.tile([P, 512], FP32, tag=f"nb{h}", name=f"bb{h}")
                for c in range(CH):
                    for C in range(4):
                        nc.tensor.matmul(
                            bb[32 * C : 32 * C + 32, :],
                            w64_sb[64 * h : 64 * h + 64, 32 * c : 32 * c + 32],
                            featH[c][64 * h : 64 * h + 64, :],
                            start=(c == 0), stop=(c == CH - 1),
                            tile_position=(64 * h, 32 * C),
                        )
                bbank.append(bb)
            # A bank: rows 32h + 4c + m = A[i, k=8c+4h+m] over free i (64).
            a32s = []
            for h in range(2):
                ab = fps.tile([P, 64], FP32, tag="psg", name=f"abank{h}")
                for c in range(CH):
                    nc.tensor.matmul(
                        ab[0:32, :],
                        w64_sb[64 * h : 64 * h + 64, 32 * c : 32 * c + 32],
                        fC16[c][64 * h : 64 * h + 64, :],
                        start=(c == 0), stop=(c == CH - 1),
                        tile_position=(64 * h, 0),
                    )
                a32 = cst.tile([32, 64], FP32, tag=f"a32_{h}", name=f"a32_{h}")
                nc.vector.tensor_copy(a32, ab[0:32, :])
                a32s.append(a32)
            # B_dup_h = bbank_h * rmask (-0.5 on min rows, 0 on abs rows), fp16;
            # then duplicated into both partition halves per C-pair (lo: C0|C1, hi: C2|C3)
            bdup = {}
            for h in range(2):
                bd = cst.tile([P, 512], FP16, tag=f"bd{h}", name=f"bd{h}")
                nc.vector.tensor_scalar(bd, bbank[h], rmask_sb[:, 0:1], None, op0=OP.mult)
                for half, nm in ((0, "lo"), (1, "hi")):
                    dd = cst.tile([P, 512], FP16, tag=f"bd{h}{nm}", name=f"bd{h}{nm}")
                    for a in range(2):
                        nc.gpsimd.dma_start(
                            out=dd[64 * a : 64 * a + 64, :],
                            in_=bd[64 * half : 64 * half + 64, :],
                        )
                    bdup[(h, half)] = dd
            # negA_h [128, NPAIR]: row 32C+4c+m, col p = -A[i=4p+C, k] (min rows)
            # replicate abank row block to all four col-group row ranges (SBUF copy via DMA)
            arep = []
            for h in range(2):
                at = cst.tile([P, 64], FP32, tag=f"arep{h}", name=f"arep{h}")
                for C in range(4):
                    nc.gpsimd.dma_start(out=at[32 * C : 32 * C + 32, :], in_=a32s[h])
                arep.append(at)
            negA = []
            for h in range(2):
                na = cst.tile([P, NPAIR], FP32, tag=f"na{h}", name=f"na{h}")
                for C in range(4):
                    nc.vector.tensor_copy(
                        na[32 * C : 32 * C + 32, :],
                        arep[h][32 * C : 32 * C + 32, C : C + 4 * (NPAIR - 1) + 1 : 4],
                    )
                nc.vector.tensor_scalar(na, na, nmask_sb[:, 0:1], None, op0=OP.mult)
                negA.append(na)

            o_raw = cst.tile([P, NG], FP32, tag="o_raw")

            def make_ads(p):
                ads = {}
                for gam in range(2):
                    g = 2 * p + gam
                    for s in range(2):
                        i = 2 * g + s
                        for c in range(CH):
                            ad = adp.tile([P, 512], FP16, tag="ad", name="ad")
                            col = fCol[c][:, i : i + 1]
                            if (s, c) in ACT_SC:
                                nc.scalar.activation(
                                    ad, featH[c], AF.Abs, bias=col, scale=-1.0
                                )
                            else:
                                nc.vector.tensor_scalar(
                                    ad, featH[c], col, None, op0=OP.min
                                )
                            ads[(gam, s, c)] = ad
                return ads

            # ---------------- phase 1 ----------------
            for p in range(NPAIR):
                ads = make_ads(p)
                banks = []
                for h in range(2):
                    bk = nps.tile([P, 512], FP32, tag=f"nb{h}", name=f"nb{h}")
                    banks.append(bk)
                # d-reduce: c-major for weight reuse across the 8 slots
                for c in range(CH):
                    for gam in range(2):
                        for s in range(2):
                            C = 2 * gam + s
                            for h in range(2):
                                nc.tensor.matmul(
                                    banks[h][32 * C : 32 * C + 32, :],
                                    w64_sb[64 * h : 64 * h + 64, 32 * c : 32 * c + 32],
                                    ads[(gam, s, c)][64 * h : 64 * h + 64, :],
                                    start=(c == 0), stop=False,
                                    tile_position=(64 * h, 32 * C),
                                )
                # B correction (adds -0.5*B on min rows, 0 on abs rows)
                for h in range(2):
                    for C in range(4):
                        nc.tensor.matmul(
                            banks[h][32 * C : 32 * C + 32, :],
                            iblk_sb[64 * h : 64 * h + 64, 32 * (C % 2) : 32 * (C % 2) + 32],
                            bdup[(h, C // 2)][64 * h : 64 * h + 64, :],
                            start=False, stop=False,
                            tile_position=(64 * h, 32 * C),
                        )
                # eraser: -+BIG at the global diagonal column j = 64*core + 4p + C,
                # selected by the per-core indicator in ebuf (window offset is
                # compile-time, the core offset lives in the data).
                for h in range(2):
                    for C in range(4):
                        off = 512 - 4 * p - C
                        nc.tensor.matmul(
                            banks[h][32 * C : 32 * C + 32, :],
                            bigw_sb[64 * h : 64 * h + 64, 32 * C : 32 * C + 32],
                            ebuf_sb[64 * h : 64 * h + 64, off : off + 512],
                            start=False, stop=True,
                            tile_position=(64 * h, 32 * C),
                        )
                # exp + accumulate over j
                for h in range(2):
                    scr = scp.tile([P, 512], FP16, tag="scr", name="scr")
                    nc.scalar.activation(
                        scr, banks[h], AF.Exp,
                        bias=negA[h][:, p : p + 1],
                        scale=scol_sb[:, 0:1],
                        accum_out=o_raw[:, 2 * p + h : 2 * p + h + 1],
                    )

            # ---------------- epilogue ----------------
            out_ps = ops.tile([NPAIR, 256], FP32, tag="out_ps")
            for h in range(2):
                nc.tensor.matmul(
                    out_ps,
                    o_raw.rearrange("q (p h) -> q h p", h=2)[:, h, :],
                    perm_sb[:, h, :],
                    start=(h == 0), stop=(h == 1),
                )
            ob_sb = cst.tile([NPAIR, 256], FP32, tag="ob")
            nc.vector.tensor_copy(ob_sb, out_ps)
            out_ob = out_h[:, INSIZE : INSIZE + K].rearrange(
                "(p gs) k -> p gs k", gs=4
            )
            nc.sync.dma_start(
                out=out_ob, in_=ob_sb.rearrange("p (gs k) -> p gs k", gs=4)
            )

    nc.finalize()
    return nc


def _consts():
    # w64: pattern c maps contract row 16m+d -> out col 4c+m, both halves
    w64 = np.zeros((2, 64, CH, 32), np.float16)
    for a in range(2):
        for c in range(CH):
            for m in range(4):
                w64[a, 16 * m : 16 * m + 16, c, 4 * c + m] = 1.0
    w64 = np.ascontiguousarray(w64.reshape(P, CH * 32))
    # iblk: identity blocks for B correction
    iblk = np.zeros((2, 64, 64), np.float16)
    for a in range(2):
        for b in range(2):
            for r in range(32):
                iblk[a, 32 * b + r, 32 * b + r] = 1.0
    iblk = np.ascontiguousarray(iblk.reshape(P, 64))
    # min-row mask by bank row
    minrow = np.ones(P, bool)
    for row in range(P):
        s, c, m = _row_sc(row)
        if (s, c) in ACT_SC:
            minrow[row] = False
    # bigw: single contract-row weight, col value -BIG on min rows else +BIG
    bigw = np.zeros((2, 64, 4, 32), np.float16)
    for a in range(2):
        for C in range(4):
            for r in range(32):
                val = -BIG if minrow[32 * C + r] else BIG
                bigw[a, 0, C, r] = val
    bigw = np.ascontiguousarray(bigw.reshape(P, 4 * 32))

    scol = np.where(minrow, 2.0, -1.0).astype(np.float32).reshape(P, 1)
    rmask = np.where(minrow, -0.5, 0.0).astype(np.float32).reshape(P, 1)
    nmask = np.where(minrow, -1.0, 0.0).astype(np.float32).reshape(P, 1)
    # perm_h: row 32C+4c+m -> col C*64 + (8c+4h+m)
    perm = np.zeros((2, P, 256), np.float32)
    for h in range(2):
        for row in range(P):
            C = row // 32
            s, c, m = _row_sc(row)
            perm[h, row, C * 64 + 8 * c + 4 * h + m] = 1.0
    return w64, iblk, bigw, scol, rmask, nmask, perm


def kernel(x, W, b):
    x = np.asarray(x, np.float32)
    W = np.asarray(W, np.float32)
    if "nc" not in _cache:
        _cache["nc"] = _build()
    nc = _cache["nc"]
    w64, iblk, bigw, scol, rmask, nmask, perm = _consts()
    xT = np.ascontiguousarray(x.T)  # [INSIZE, N]
    # wt[c, p, t, kcol] = W[128c + kcol, 128t + p]
    wt = np.ascontiguousarray(
        W.reshape(CH, P, NT, P).transpose(0, 3, 2, 1).astype(np.float16)
    )
    in_maps = []
    for cidx in range(NCORES):
        xs = np.ascontiguousarray(x[NL * cidx : NL * (cidx + 1)])
        xte = np.concatenate([xT, xs.T], axis=1).astype(np.float16)  # [512, 576]
        xte_t = np.ascontiguousarray(xte.reshape(NT, P, 576))
        ebuf = np.zeros((P, 1024), np.float16)
        ebuf[0, 512 + NL * cidx] = 1.0
        ebuf[64, 512 + NL * cidx] = 1.0
        in_maps.append({
            "xte": xte_t, "wt": wt, "w64": w64, "iblk": iblk,
            "bigw": bigw, "ebuf": ebuf, "scol": scol, "rmask": rmask,
            "nmask": nmask, "perm": perm, "xs": xs,
        })
    res = run_bass_kernel_spmd(
        nc, in_maps, core_ids=list(range(NCORES)), trace=TRACE
    )
    _cache["last_results"] = res
    return np.ascontiguousarray(
        np.concatenate([res.results[c]["out"] for c in range(NCORES)], axis=0)
    )


# revision 14
# speedup vs baseline: 1.0376x; 1.0154x over previous
"""Trainium2 Bass kernel for nn_MinibatchDiscriminator (N=512, INSIZE=512, K=64, D=16).

Per core (row-shard of 64 i's, full j range), fp16 pipeline:
  feat = x @ W.T computed as featH chunks [128=(8k x 16d), 512 j] fp16
  (bias b cancels in all pairwise differences and is dropped).
  Per group of 2 i's: 16 elementwise absdiff-ish ops (13 DVE min, 3 ACT abs),
  d-reduction via 64x32-tiled PE matmuls into 2 PSUM banks per pair of
  groups (row-tile h -> bank h), B-correction + diagonal eraser as tiled
  matmuls in the same mode, then one ACT exp+accum per bank.
  o_b rows gathered via two permutation matmuls at the end.
"""
import sys

import numpy as np

sys.path.insert(0, "/opt/trn_rl_repo")

import concourse.bass as bass
import concourse.tile as tile
from concourse import bacc, mybir
from concourse.bass_utils import run_bass_kernel_spmd

AF = mybir.ActivationFunctionType
OP = mybir.AluOpType
FP32 = mybir.dt.float32
FP16 = mybir.dt.float16

N, INSIZE, K, D = 512, 512, 64, 16
KD = K * D
NCORES = 8
NL = N // NCORES          # 64 rows per core
P = 128
CH = KD // P              # 8 chunks of (8k x 16d)
NT = INSIZE // P          # 4 contraction tiles
NG = NL // 2              # 32 groups of 2 rows
NPAIR = NG // 2           # 16 pairs of groups (2 banks each)
ACT_SC = {(0, 3), (1, 3), (0, 7)}   # (s, c) absdiffs on ScalarE (abs rows)
BIG = 200.0

TRACE = False
_cache = {}


def _row_sc(row):
    """bank row (within [128]) -> (s, c, m) given col-group C=row//32."""
    C, r = row // 32, row % 32
    return C % 2, r // 4, r % 4   # s, c, m


def _build():
    nc = bacc.Bacc("TRN2", target_bir_lowering=False)
    xte_h = nc.dram_tensor("xte", [NT, P, 576], FP16, kind="ExternalInput").ap()
    wt_h = nc.dram_tensor("wt", [CH, P, NT, P], FP16, kind="ExternalInput").ap()
    w64_h = nc.dram_tensor("w64", [P, CH * 32], FP16, kind="ExternalInput").ap()
    iblk_h = nc.dram_tensor("iblk", [P, 64], FP16, kind="ExternalInput").ap()
    bigw_h = nc.dram_tensor("bigw", [P, 4 * 32], FP16, kind="ExternalInput").ap()
    ebuf_h = nc.dram_tensor("ebuf", [P, 1024], FP16, kind="ExternalInput").ap()
    scol_h = nc.dram_tensor("scol", [P, 1], FP32, kind="ExternalInput").ap()
    rmask_h = nc.dram_tensor("rmask", [P, 1], FP32, kind="ExternalInput").ap()
    nmask_h = nc.dram_tensor("nmask", [P, 1], FP32, kind="ExternalInput").ap()
    perm_h = nc.dram_tensor("perm", [2, P, 256], FP32, kind="ExternalInput").ap()
    xs_h = nc.dram_tensor("xs", [NL, INSIZE], FP32, kind="ExternalInput").ap()
    out_h = nc.dram_tensor("out", [NL, INSIZE + K], FP32, kind="ExternalOutput").ap()

    with tile.TileContext(nc) as tc:
        with (
            tc.tile_pool(name="cst", bufs=1) as cst,
            tc.tile_pool(name="inp", bufs=1) as inp,
            tc.tile_pool(name="ad", bufs=84) as adp,
            tc.tile_pool(name="scr", bufs=4) as scp,
            tc.tile_pool(name="fps", bufs=1, space="PSUM") as fps,
            tc.tile_pool(name="fp2", bufs=2, space="PSUM") as fp2,
            tc.tile_pool(name="nps", bufs=2, space="PSUM") as nps,
            tc.tile_pool(name="ops", bufs=1, space="PSUM") as ops,
        ):
            # ---------------- input DMAs ----------------
            xte_sb = []
            for t in range(NT):
                s = inp.tile([P, 576], FP16, tag=f"xte{t}", name=f"xte{t}")
                nc.sync.dma_start(out=s, in_=xte_h[t])
                xte_sb.append(s)
            wt_sb = []
            for c in range(CH):
                s = inp.tile([P, NT, P], FP16, tag=f"wt{c}", name=f"wt{c}")
                nc.sync.dma_start(out=s, in_=wt_h[c])
                wt_sb.append(s)
            w64_sb = cst.tile([P, CH * 32], FP16, tag="w64")
            nc.gpsimd.dma_start(out=w64_sb, in_=w64_h)
            iblk_sb = cst.tile([P, 64], FP16, tag="iblk")
            nc.gpsimd.dma_start(out=iblk_sb, in_=iblk_h)
            bigw_sb = cst.tile([P, 4 * 32], FP16, tag="bigw")
            nc.gpsimd.dma_start(out=bigw_sb, in_=bigw_h)
            ebuf_sb = cst.tile([P, 1024], FP16, tag="ebuf")
            nc.gpsimd.dma_start(out=ebuf_sb, in_=ebuf_h)
            scol_sb = cst.tile([P, 1], FP32, tag="scol")
            nc.gpsimd.dma_start(out=scol_sb, in_=scol_h)
            rmask_sb = cst.tile([P, 1], FP32, tag="rmask")
            nc.gpsimd.dma_start(out=rmask_sb, in_=rmask_h)
            nmask_sb = cst.tile([P, 1], FP32, tag="nmask")
            nc.gpsimd.dma_start(out=nmask_sb, in_=nmask_h)
            perm_sb = cst.tile([P, 2, 256], FP32, tag="perm")
            nc.gpsimd.dma_start(out=perm_sb, in_=perm_h.rearrange("h p f -> p h f"))
            # x passthrough
            nc.gpsimd.dma_start(out=out_h[:, 0:INSIZE], in_=xs_h)

            # ---------------- featH / fCol per chunk ----------------
            featH, fCol, fC16 = [], [], []
            early_ads = {}
            for c in range(CH):
                psf = fp2.tile([P, 512], FP32, tag="psf", name="psf")
                for t in range(NT):
                    nc.tensor.matmul(
                        psf, wt_sb[c][:, t, :], xte_sb[t][:, 0:512],
                        start=(t == 0), stop=(t == NT - 1),
                    )
                psg = fps.tile([P, 64], FP32, tag="psg", name="psg")
                for t in range(NT):
                    nc.tensor.matmul(
                        psg, wt_sb[c][:, t, :], xte_sb[t][:, 512:576],
                        start=(t == 0), stop=(t == NT - 1),
                    )
                fh = inp.tile([P, 512], FP16, tag=f"fh{c}", name=f"fh{c}")
                nc.scalar.copy(fh, psf)
                featH.append(fh)
                fc = inp.tile([P, 64], FP32, tag=f"fc{c}", name=f"fc{c}")
                nc.vector.tensor_copy(fc, psg)
                fCol.append(fc)
                f16 = inp.tile([P, 64], FP16, tag=f"f16_{c}", name=f"f16_{c}")
                nc.vector.tensor_copy(f16, psg)
                fC16.append(f16)
                for p in range(2):
                    for gam in range(2):
                        for s in range(2):
                            i = 2 * (2 * p + gam) + s
                            ad = adp.tile([P, 512], FP16, tag="ad", name="ad")
                            col = fc[:, i : i + 1]
                            if (s, c) in ACT_SC:
                                nc.scalar.activation(
                                    ad, fh, AF.Abs, bias=col, scale=-1.0
                                )
                            else:
                                nc.vector.tensor_scalar(
                                    ad, fh, col, None, op0=OP.min
                                )
                            early_ads[(p, gam, s, c)] = ad


            # ---------------- B and A tables (tiled reduce of featH/fCol) ----------------
            # B banks: one per h; rows 32C+4c+m = B[k=8c+4h+m, j] for every C.
            bbank = []
            for h in range(2):
                bb = nps.tile([P, 512], FP32, tag=f"nb{h}", name=f"bb{h}")
                for c in range(CH):
                    for C in range(4):
                        nc.tensor.matmul(
                            bb[32 * C : 32 * C + 32, :],
                            w64_sb[64 * h : 64 * h + 64, 32 * c : 32 * c + 32],
                            featH[c][64 * h : 64 * h + 64, :],
                            start=(c == 0), stop=(c == CH - 1),
                            tile_position=(64 * h, 32 * C),
                        )
                bbank.append(bb)
            # A bank: rows 32h + 4c + m = A[i, k=8c+4h+m] over free i (64).
            a32s = []
            for h in range(2):
                ab = fps.tile([P, 64], FP32, tag="psg", name=f"abank{h}")
                for c in range(CH):
                    nc.tensor.matmul(
                        ab[0:32, :],
                        w64_sb[64 * h : 64 * h + 64, 32 * c : 32 * c + 32],
                        fC16[c][64 * h : 64 * h + 64, :],
                        start=(c == 0), stop=(c == CH - 1),
                        tile_position=(64 * h, 0),
                    )
                a32 = cst.tile([32, 64], FP32, tag=f"a32_{h}", name=f"a32_{h}")
                nc.vector.tensor_copy(a32, ab[0:32, :])
                a32s.append(a32)
            # B_dup_h = bbank_h * rmask (-0.5 on min rows, 0 on abs rows), fp16;
            # then duplicated into both partition halves per C-pair (lo: C0|C1, hi: C2|C3)
            bdup = {}
            for h in range(2):
                bd = cst.tile([P, 512], FP16, tag=f"bd{h}", name=f"bd{h}")
                nc.vector.tensor_scalar(bd, bbank[h], rmask_sb[:, 0:1], None, op0=OP.mult)
                for half, nm in ((0, "lo"), (1, "hi")):
                    dd = cst.tile([P, 512], FP16, tag=f"bd{h}{nm}", name=f"bd{h}{nm}")
                    for a in range(2):
                        nc.gpsimd.dma_start(
                            out=dd[64 * a : 64 * a + 64, :],
                            in_=bd[64 * half : 64 * half + 64, :],
                        )
                    bdup[(h, half)] = dd
            # negA_h [128, NPAIR]: row 32C+4c+m, col p = -A[i=4p+C, k] (min rows)
            # replicate abank row block to all four col-group row ranges (SBUF copy via DMA)
            arep = []
            for h in range(2):
                at = cst.tile([P, 64], FP32, tag=f"arep{h}", name=f"arep{h}")
                for C in range(4):
                    nc.gpsimd.dma_start(out=at[32 * C : 32 * C + 32, :], in_=a32s[h])
                arep.append(at)
            negA = []
            for h in range(2):
                na = cst.tile([P, NPAIR], FP32, tag=f"na{h}", name=f"na{h}")
                for C in range(4):
                    nc.vector.tensor_copy(
                        na[32 * C : 32 * C + 32, :],
                        arep[h][32 * C : 32 * C + 32, C : C + 4 * (NPAIR - 1) + 1 : 4],
                    )
                nc.vector.tensor_scalar(na, na, nmask_sb[:, 0:1], None, op0=OP.mult)
                negA.append(na)

            o_raw = cst.tile([P, NG], FP32, tag="o_raw")

            def make_ads(p):
                ads = {}
                for gam in range(2):
                    g = 2 * p + gam
                    for s in range(2):
                        i = 2 * g + s
                        for c in range(CH):
                            ad = adp.tile([P, 512], FP16, tag="ad", name="ad")
                            col = fCol[c][:, i : i + 1]
                            if (s, c) in ACT_SC:
                                nc.scalar.activation(
                                    ad, featH[c], AF.Abs, bias=col, scale=-1.0
                                )
                            else:
                                nc.vector.tensor_scalar(
                                    ad, featH[c], col, None, op0=OP.min
                                )
                            ads[(gam, s, c)] = ad
                return ads

            # ---------------- phase 1 ----------------
            for p in range(NPAIR):
                if p < 2:
                    ads = {k[1:]: v for k, v in early_ads.items() if k[0] == p}
                else:
                    ads = make_ads(p)
                banks = []
                for h in range(2):
                    bk = nps.tile([P, 512], FP32, tag=f"nb{h}", name=f"nb{h}")
                    banks.append(bk)
                # d-reduce: c-major for weight reuse across the 8 slots
                for c in range(CH):
                    for gam in range(2):
                        for s in range(2):
                            C = 2 * gam + s
                            for h in range(2):
                                nc.tensor.matmul(
                                    banks[h][32 * C : 32 * C + 32, :],
                                    w64_sb[64 * h : 64 * h + 64, 32 * c : 32 * c + 32],
                                    ads[(gam, s, c)][64 * h : 64 * h + 64, :],
                                    start=(c == 0), stop=False,
                                    tile_position=(64 * h, 32 * C),
                                )
                # B correction (adds -0.5*B on min rows, 0 on abs rows)
                for h in range(2):
                    for C in range(4):
                        nc.tensor.matmul(
                            banks[h][32 * C : 32 * C + 32, :],
                            iblk_sb[64 * h : 64 * h + 64, 32 * (C % 2) : 32 * (C % 2) + 32],
                            bdup[(h, C // 2)][64 * h : 64 * h + 64, :],
                            start=False, stop=False,
                            tile_position=(64 * h, 32 * C),
                        )
                # eraser: -+BIG at the global diagonal column j = 64*core + 4p + C,
                # selected by the per-core indicator in ebuf (window offset is
                # compile-time, the core offset lives in the data).
                for h in range(2):
                    for C in range(4):
                        off = 512 - 4 * p - C
                        nc.tensor.matmul(
                            banks[h][32 * C : 32 * C + 32, :],
                            bigw_sb[64 * h : 64 * h + 64, 32 * C : 32 * C + 32],
                            ebuf_sb[64 * h : 64 * h + 64, off : off + 512],
                            start=False, stop=True,
                            tile_position=(64 * h, 32 * C),
                        )
                # exp + accumulate over j
                for h in range(2):
                    scr = scp.tile([P, 512], FP16, tag="scr", name="scr")
                    nc.scalar.activation(
                        scr, banks[h], AF.Exp,
                        bias=negA[h][:, p : p + 1],
                        scale=scol_sb[:, 0:1],
                        accum_out=o_raw[:, 2 * p + h : 2 * p + h + 1],
                    )

            # ---------------- epilogue ----------------
            out_ps = ops.tile([NPAIR, 256], FP32, tag="out_ps")
            for h in range(2):
                nc.tensor.matmul(
                    out_ps,
                    o_raw.rearrange("q (p h) -> q h p", h=2)[:, h, :],
                    perm_sb[:, h, :],
                    start=(h == 0), stop=(h == 1),
                )
            ob_sb = cst.tile([NPAIR, 256], FP32, tag="ob")
            nc.vector.tensor_copy(ob_sb, out_ps)
            out_ob = out_h[:, INSIZE : INSIZE + K].rearrange(
                "(p gs) k -> p gs k", gs=4
            )
            nc.sync.dma_start(
                out=out_ob, in_=ob_sb.rearrange("p (gs k) -> p gs k", gs=4)
            )

    nc.finalize()
    return nc


def _consts():
    # w64: pattern c maps contract row 16m+d -> out col 4c+m, both halves
    w64 = np.zeros((2, 64, CH, 32), np.float16)
    for a in range(2):
        for c in range(CH):
            for m in range(4):
                w64[a, 16 * m : 16 * m + 16, c, 4 * c + m] = 1.0
    w64 = np.ascontiguousarray(w64.reshape(P, CH * 32))
    # iblk: identity blocks for B correction
    iblk = np.zeros((2, 64, 64), np.float16)
    for a in range(2):
        for b in range(2):
            for r in range(32):
                iblk[a, 32 * b + r, 32 * b + r] = 1.0
    iblk = np.ascontiguousarray(iblk.reshape(P, 64))
    # min-row mask by bank row
    minrow = np.ones(P, bool)
    for row in range(P):
        s, c, m = _row_sc(row)
        if (s, c) in ACT_SC:
            minrow[row] = False
    # bigw: single contract-row weight, col value -BIG on min rows else +BIG
    bigw = np.zeros((2, 64, 4, 32), np.float16)
    for a in range(2):
        for C in range(4):
            for r in range(32):
                val = -BIG if minrow[32 * C + r] else BIG
                bigw[a, 0, C, r] = val
    bigw = np.ascontiguousarray(bigw.reshape(P, 4 * 32))

    scol = np.where(minrow, 2.0, -1.0).astype(np.float32).reshape(P, 1)
    rmask = np.where(minrow, -0.5, 0.0).astype(np.float32).reshape(P, 1)
    nmask = np.where(minrow, -1.0, 0.0).astype(np.float32).reshape(P, 1)
    # perm_h: row 32C+4c+m -> col C*64 + (8c+4h+m)
    perm = np.zeros((2, P, 256), np.float32)
    for h in range(2):
        for row in range(P):
            C = row // 32
            s, c, m = _row_sc(row)
            perm[h, row, C * 64 + 8 * c + 4 * h + m] = 1.0
    return w64, iblk, bigw, scol, rmask, nmask, perm


def kernel(x, W, b):
    x = np.asarray(x, np.float32)
    W = np.asarray(W, np.float32)
    if "nc" not in _cache:
        _cache["nc"] = _build()
    nc = _cache["nc"]
    w64, iblk, bigw, scol, rmask, nmask, perm = _consts()
    xT = np.ascontiguousarray(x.T)  # [INSIZE, N]
    # wt[c, p, t, kcol] = W[128c + kcol, 128t + p]
    wt = np.ascontiguousarray(
        W.reshape(CH, P, NT, P).transpose(0, 3, 2, 1).astype(np.float16)
    )
    in_maps = []
    for cidx in range(NCORES):
        xs = np.ascontiguousarray(x[NL * cidx : NL * (cidx + 1)])
        xte = np.concatenate([xT, xs.T], axis=1).astype(np.float16)  # [512, 576]
        xte_t = np.ascontiguousarray(xte.reshape(NT, P, 576))
        ebuf = np.zeros((P, 1024), np.float16)
        ebuf[0, 512 + NL * cidx] = 1.0
        ebuf[64, 512 + NL * cidx] = 1.0
        in_maps.append({
            "xte": xte_t, "wt": wt, "w64": w64, "iblk": iblk,
            "bigw": bigw, "ebuf": ebuf, "scol": scol, "rmask": rmask,
            "nmask": nmask, "perm": perm, "xs": xs,
        })
    res = run_bass_kernel_spmd(
        nc, in_maps, core_ids=list(range(NCORES)), trace=TRACE
    )
    _cache["last_results"] = res
    return np.ascontiguousarray(
        np.concatenate([res.results[c]["out"] for c in range(NCORES)], axis=0)
    )


# revision 15
# speedup vs baseline: 1.0773x; 1.0382x over previous
"""Trainium2 Bass kernel for nn_MinibatchDiscriminator (N=512, INSIZE=512, K=64, D=16).

Per core (row-shard of 64 i's, full j range), fp16 pipeline:
  feat = x @ W.T computed as featH chunks [128=(8k x 16d), 512 j] fp16
  (bias b cancels in all pairwise differences and is dropped).
  Per group of 2 i's: 16 elementwise absdiff-ish ops (13 DVE min, 3 ACT abs),
  d-reduction via 64x32-tiled PE matmuls into 2 PSUM banks per pair of
  groups (row-tile h -> bank h), B-correction + diagonal eraser as tiled
  matmuls in the same mode, then one ACT exp+accum per bank.
  o_b rows gathered via two permutation matmuls at the end.
"""
import sys

import numpy as np

sys.path.insert(0, "/opt/trn_rl_repo")

import concourse.bass as bass
import concourse.tile as tile
from concourse import bacc, mybir
from concourse.bass_utils import run_bass_kernel_spmd

AF = mybir.ActivationFunctionType
OP = mybir.AluOpType
FP32 = mybir.dt.float32
FP16 = mybir.dt.float16

N, INSIZE, K, D = 512, 512, 64, 16
KD = K * D
NCORES = 8
NL = N // NCORES          # 64 rows per core
P = 128
CH = KD // P              # 8 chunks of (8k x 16d)
NT = INSIZE // P          # 4 contraction tiles
NG = NL // 2              # 32 groups of 2 rows
NPAIR = NG // 2           # 16 pairs of groups (2 banks each)
ACT_SC = {(0, 3), (1, 3), (0, 7)}   # (s, c) absdiffs on ScalarE (abs rows)
BIG = 200.0

TRACE = False
_cache = {}


def _row_sc(row):
    """bank row (within [128]) -> (s, c, m) given col-group C=row//32."""
    C, r = row // 32, row % 32
    return C % 2, r // 4, r % 4   # s, c, m


def _build():
    nc = bacc.Bacc("TRN2", target_bir_lowering=False)
    xte_h = nc.dram_tensor("xte", [NT, P, 576], FP16, kind="ExternalInput").ap()
    wt_h = nc.dram_tensor("wt", [CH, P, NT, P], FP16, kind="ExternalInput").ap()
    w64_h = nc.dram_tensor("w64", [P, CH * 32], FP16, kind="ExternalInput").ap()
    iblk_h = nc.dram_tensor("iblk", [P, 64], FP16, kind="ExternalInput").ap()
    bigw_h = nc.dram_tensor("bigw", [P, 4 * 32], FP16, kind="ExternalInput").ap()
    ebuf_h = nc.dram_tensor("ebuf", [P, 1024], FP16, kind="ExternalInput").ap()
    scol_h = nc.dram_tensor("scol", [P, 1], FP32, kind="ExternalInput").ap()
    rmask_h = nc.dram_tensor("rmask", [P, 1], FP32, kind="ExternalInput").ap()
    nmask_h = nc.dram_tensor("nmask", [P, 1], FP32, kind="ExternalInput").ap()
    perm_h = nc.dram_tensor("perm", [2, P, 256], FP32, kind="ExternalInput").ap()
    xs_h = nc.dram_tensor("xs", [NL, INSIZE], FP32, kind="ExternalInput").ap()
    out_h = nc.dram_tensor("out", [NL, INSIZE + K], FP32, kind="ExternalOutput").ap()

    with tile.TileContext(nc) as tc:
        with (
            tc.tile_pool(name="cst", bufs=1) as cst,
            tc.tile_pool(name="inp", bufs=1) as inp,
            tc.tile_pool(name="ad", bufs=84) as adp,
            tc.tile_pool(name="scr", bufs=4) as scp,
            tc.tile_pool(name="fps", bufs=1, space="PSUM") as fps,
            tc.tile_pool(name="fp2", bufs=2, space="PSUM") as fp2,
            tc.tile_pool(name="nps", bufs=2, space="PSUM") as nps,
            tc.tile_pool(name="ops", bufs=1, space="PSUM") as ops,
        ):
            # ---------------- input DMAs ----------------
            xte_sb = []
            for t in range(NT):
                s = inp.tile([P, 576], FP16, tag=f"xte{t}", name=f"xte{t}")
                nc.sync.dma_start(out=s, in_=xte_h[t])
                xte_sb.append(s)
            wt_sb = []
            for c in range(CH):
                s = inp.tile([P, NT, P], FP16, tag=f"wt{c}", name=f"wt{c}")
                nc.sync.dma_start(out=s, in_=wt_h[c])
                wt_sb.append(s)
            w64_sb = cst.tile([P, CH * 32], FP16, tag="w64")
            nc.gpsimd.dma_start(out=w64_sb, in_=w64_h)
            iblk_sb = cst.tile([P, 64], FP16, tag="iblk")
            nc.gpsimd.dma_start(out=iblk_sb, in_=iblk_h)
            bigw_sb = cst.tile([P, 4 * 32], FP16, tag="bigw")
            nc.gpsimd.dma_start(out=bigw_sb, in_=bigw_h)
            ebuf_sb = cst.tile([P, 1024], FP16, tag="ebuf")
            nc.gpsimd.dma_start(out=ebuf_sb, in_=ebuf_h)
            scol_sb = cst.tile([P, 1], FP32, tag="scol")
            nc.gpsimd.dma_start(out=scol_sb, in_=scol_h)
            rmask_sb = cst.tile([P, 1], FP32, tag="rmask")
            nc.gpsimd.dma_start(out=rmask_sb, in_=rmask_h)
            nmask_sb = cst.tile([P, 1], FP32, tag="nmask")
            nc.gpsimd.dma_start(out=nmask_sb, in_=nmask_h)

            # ---------------- featH / fCol per chunk ----------------
            featH, fCol, fC16 = [], [], []
            early_ads = {}
            for c in range(CH):
                psf = fp2.tile([P, 512], FP32, tag="psf", name="psf")
                for t in range(NT):
                    nc.tensor.matmul(
                        psf, wt_sb[c][:, t, :], xte_sb[t][:, 0:512],
                        start=(t == 0), stop=(t == NT - 1),
                    )
                psg = fps.tile([P, 64], FP32, tag="psg", name="psg")
                for t in range(NT):
                    nc.tensor.matmul(
                        psg, wt_sb[c][:, t, :], xte_sb[t][:, 512:576],
                        start=(t == 0), stop=(t == NT - 1),
                    )
                fh = inp.tile([P, 512], FP16, tag=f"fh{c}", name=f"fh{c}")
                nc.scalar.copy(fh, psf)
                featH.append(fh)
                fc = inp.tile([P, 64], FP32, tag=f"fc{c}", name=f"fc{c}")
                nc.vector.tensor_copy(fc, psg)
                fCol.append(fc)
                f16 = inp.tile([P, 64], FP16, tag=f"f16_{c}", name=f"f16_{c}")
                nc.vector.tensor_copy(f16, psg)
                fC16.append(f16)
                for p in range(2):
                    for gam in range(2):
                        for s in range(2):
                            i = 2 * (2 * p + gam) + s
                            ad = adp.tile([P, 512], FP16, tag="ad", name="ad")
                            col = fc[:, i : i + 1]
                            if (s, c) in ACT_SC:
                                nc.scalar.activation(
                                    ad, fh, AF.Abs, bias=col, scale=-1.0
                                )
                            else:
                                nc.vector.tensor_scalar(
                                    ad, fh, col, None, op0=OP.min
                                )
                            early_ads[(p, gam, s, c)] = ad


            # ---------------- B and A tables (tiled reduce of featH/fCol) ----------------
            # B banks: one per h; rows 32C+4c+m = B[k=8c+4h+m, j] for every C.
            bbank = []
            for h in range(2):
                bb = nps.tile([P, 512], FP32, tag=f"nb{h}", name=f"bb{h}")
                for c in range(CH):
                    for C in range(4):
                        nc.tensor.matmul(
                            bb[32 * C : 32 * C + 32, :],
                            w64_sb[64 * h : 64 * h + 64, 32 * c : 32 * c + 32],
                            featH[c][64 * h : 64 * h + 64, :],
                            start=(c == 0), stop=(c == CH - 1),
                            tile_position=(64 * h, 32 * C),
                        )
                bbank.append(bb)
            # A bank: rows 32h + 4c + m = A[i, k=8c+4h+m] over free i (64).
            a32s = []
            for h in range(2):
                ab = fps.tile([P, 64], FP32, tag="psg", name=f"abank{h}")
                for c in range(CH):
                    nc.tensor.matmul(
                        ab[0:32, :],
                        w64_sb[64 * h : 64 * h + 64, 32 * c : 32 * c + 32],
                        fC16[c][64 * h : 64 * h + 64, :],
                        start=(c == 0), stop=(c == CH - 1),
                        tile_position=(64 * h, 0),
                    )
                a32 = cst.tile([32, 64], FP32, tag=f"a32_{h}", name=f"a32_{h}")
                nc.vector.tensor_copy(a32, ab[0:32, :])
                a32s.append(a32)
            # B_dup_h = bbank_h * rmask (-0.5 on min rows, 0 on abs rows), fp16;
            # then duplicated into both partition halves per C-pair (lo: C0|C1, hi: C2|C3)
            bdup = {}
            for h in range(2):
                bd = cst.tile([P, 512], FP16, tag=f"bd{h}", name=f"bd{h}")
                nc.vector.tensor_scalar(bd, bbank[h], rmask_sb[:, 0:1], None, op0=OP.mult)
                for half, nm in ((0, "lo"), (1, "hi")):
                    dd = cst.tile([P, 512], FP16, tag=f"bd{h}{nm}", name=f"bd{h}{nm}")
                    for a in range(2):
                        nc.gpsimd.dma_start(
                            out=dd[64 * a : 64 * a + 64, :],
                            in_=bd[64 * half : 64 * half + 64, :],
                        )
                    bdup[(h, half)] = dd
            # negA_h [128, NPAIR]: row 32C+4c+m, col p = -A[i=4p+C, k] (min rows)
            # replicate abank row block to all four col-group row ranges (SBUF copy via DMA)
            arep = []
            for h in range(2):
                at = cst.tile([P, 64], FP32, tag=f"arep{h}", name=f"arep{h}")
                for C in range(4):
                    nc.sync.dma_start(out=at[32 * C : 32 * C + 32, :], in_=a32s[h])
                arep.append(at)
            negA = []
            for h in range(2):
                na = cst.tile([P, NPAIR], FP32, tag=f"na{h}", name=f"na{h}")
                for C in range(4):
                    nc.vector.tensor_copy(
                        na[32 * C : 32 * C + 32, :],
                        arep[h][32 * C : 32 * C + 32, C : C + 4 * (NPAIR - 1) + 1 : 4],
                    )
                nc.vector.tensor_scalar(na, na, nmask_sb[:, 0:1], None, op0=OP.mult)
                negA.append(na)

            o_raw = cst.tile([P, NG], FP32, tag="o_raw")
            perm_pre = cst.tile([P, 2, 256], FP32, tag="perm")
            nc.gpsimd.dma_start(out=perm_pre, in_=perm_h.rearrange("h p f -> p h f"))
            nc.gpsimd.dma_start(out=out_h[:, 0:INSIZE], in_=xs_h)

            def make_ads(p):
                ads = {}
                for c in range(CH):
                    for gam in range(2):
                        for s in range(2):
                            i = 2 * (2 * p + gam) + s
                            ad = adp.tile([P, 512], FP16, tag="ad", name="ad")
                            col = fCol[c][:, i : i + 1]
                            if (s, c) in ACT_SC:
                                nc.scalar.activation(
                                    ad, featH[c], AF.Abs, bias=col, scale=-1.0
                                )
                            else:
                                nc.vector.tensor_scalar(
                                    ad, featH[c], col, None, op0=OP.min
                                )
                            ads[(gam, s, c)] = ad
                return ads

            # ---------------- phase 1 ----------------
            for p in range(NPAIR):
                if p < 2:
                    ads = {k[1:]: v for k, v in early_ads.items() if k[0] == p}
                else:
                    ads = make_ads(p)
                banks = []
                for h in range(2):
                    bk = nps.tile([P, 512], FP32, tag=f"nb{h}", name=f"nb{h}")
                    banks.append(bk)
                # d-reduce: c-major for weight reuse across the 8 slots
                for c in range(CH):
                    for gam in range(2):
                        for s in range(2):
                            C = 2 * gam + s
                            for h in range(2):
                                nc.tensor.matmul(
                                    banks[h][32 * C : 32 * C + 32, :],
                                    w64_sb[64 * h : 64 * h + 64, 32 * c : 32 * c + 32],
                                    ads[(gam, s, c)][64 * h : 64 * h + 64, :],
                                    start=(c == 0), stop=False,
                                    tile_position=(64 * h, 32 * C),
                                )
                # B correction (adds -0.5*B on min rows, 0 on abs rows)
                for h in range(2):
                    for C in range(4):
                        nc.tensor.matmul(
                            banks[h][32 * C : 32 * C + 32, :],
                            iblk_sb[64 * h : 64 * h + 64, 32 * (C % 2) : 32 * (C % 2) + 32],
                            bdup[(h, C // 2)][64 * h : 64 * h + 64, :],
                            start=False, stop=False,
                            tile_position=(64 * h, 32 * C),
                        )
                # eraser: -+BIG at the global diagonal column j = 64*core + 4p + C,
                # selected by the per-core indicator in ebuf (window offset is
                # compile-time, the core offset lives in the data).
                for h in range(2):
                    for C in range(4):
                        off = 512 - 4 * p - C
                        nc.tensor.matmul(
                            banks[h][32 * C : 32 * C + 32, :],
                            bigw_sb[64 * h : 64 * h + 64, 32 * C : 32 * C + 32],
                            ebuf_sb[64 * h : 64 * h + 64, off : off + 512],
                            start=False, stop=True,
                            tile_position=(64 * h, 32 * C),
                        )
                # exp + accumulate over j
                for h in range(2):
                    scr = scp.tile([P, 512], FP16, tag="scr", name="scr")
                    nc.scalar.activation(
                        scr, banks[h], AF.Exp,
                        bias=negA[h][:, p : p + 1],
                        scale=scol_sb[:, 0:1],
                        accum_out=o_raw[:, 2 * p + h : 2 * p + h + 1],
                    )

            # ---------------- epilogue ----------------
            perm_sb = perm_pre
            out_ps = ops.tile([NPAIR, 256], FP32, tag="out_ps")
            for h in range(2):
                nc.tensor.matmul(
                    out_ps,
                    o_raw.rearrange("q (p h) -> q h p", h=2)[:, h, :],
                    perm_sb[:, h, :],
                    start=(h == 0), stop=(h == 1),
                )
            ob_sb = cst.tile([NPAIR, 256], FP32, tag="ob")
            nc.vector.tensor_copy(ob_sb, out_ps)
            out_ob = out_h[:, INSIZE : INSIZE + K].rearrange(
                "(p gs) k -> p gs k", gs=4
            )
            nc.sync.dma_start(
                out=out_ob, in_=ob_sb.rearrange("p (gs k) -> p gs k", gs=4)
            )

    nc.finalize()
    return nc


def _consts():
    # w64: pattern c maps contract row 16m+d -> out col 4c+m, both halves
    w64 = np.zeros((2, 64, CH, 32), np.float16)
    for a in range(2):
        for c in range(CH):
            for m in range(4):
                w64[a, 16 * m : 16 * m + 16, c, 4 * c + m] = 1.0
    w64 = np.ascontiguousarray(w64.reshape(P, CH * 32))
    # iblk: identity blocks for B correction
    iblk = np.zeros((2, 64, 64), np.float16)
    for a in range(2):
        for b in range(2):
            for r in range(32):
                iblk[a, 32 * b + r, 32 * b + r] = 1.0
    iblk = np.ascontiguousarray(iblk.reshape(P, 64))
    # min-row mask by bank row
    minrow = np.ones(P, bool)
    for row in range(P):
        s, c, m = _row_sc(row)
        if (s, c) in ACT_SC:
            minrow[row] = False
    # bigw: single contract-row weight, col value -BIG on min rows else +BIG
    bigw = np.zeros((2, 64, 4, 32), np.float16)
    for a in range(2):
        for C in range(4):
            for r in range(32):
                val = -BIG if minrow[32 * C + r] else BIG
                bigw[a, 0, C, r] = val
    bigw = np.ascontiguousarray(bigw.reshape(P, 4 * 32))

    scol = np.where(minrow, 2.0, -1.0).astype(np.float32).reshape(P, 1)
    rmask = np.where(minrow, -0.5, 0.0).astype(np.float32).reshape(P, 1)
    nmask = np.where(minrow, -1.0, 0.0).astype(np.float32).reshape(P, 1)
    # perm_h: row 32C+4c+m -> col C*64 + (8c+4h+m)
    perm = np.zeros((2, P, 256), np.float32)
    for h in range(2):
        for row in range(P):
            C = row // 32
            s, c, m = _row_sc(row)
            perm[h, row, C * 64 + 8 * c + 4 * h + m] = 1.0
    return w64, iblk, bigw, scol, rmask, nmask, perm


def kernel(x, W, b):
    x = np.asarray(x, np.float32)
    W = np.asarray(W, np.float32)
    if "nc" not in _cache:
        _cache["nc"] = _build()
    nc = _cache["nc"]
    w64, iblk, bigw, scol, rmask, nmask, perm = _consts()
    xT = np.ascontiguousarray(x.T)  # [INSIZE, N]
    # wt[c, p, t, kcol] = W[128c + kcol, 128t + p]
    wt = np.ascontiguousarray(
        W.reshape(CH, P, NT, P).transpose(0, 3, 2, 1).astype(np.float16)
    )
    in_maps = []
    for cidx in range(NCORES):
        xs = np.ascontiguousarray(x[NL * cidx : NL * (cidx + 1)])
        xte = np.concatenate([xT, xs.T], axis=1).astype(np.float16)  # [512, 576]
        xte_t = np.ascontiguousarray(xte.reshape(NT, P, 576))
        ebuf = np.zeros((P, 1024), np.float16)
        ebuf[0, 512 + NL * cidx] = 1.0
        ebuf[64, 512 + NL * cidx] = 1.0
        in_maps.append({
            "xte": xte_t, "wt": wt, "w64": w64, "iblk": iblk,
            "bigw": bigw, "ebuf": ebuf, "scol": scol, "rmask": rmask,
            "nmask": nmask, "perm": perm, "xs": xs,
        })
    res = run_bass_kernel_spmd(
        nc, in_maps, core_ids=list(range(NCORES)), trace=TRACE
    )
    _cache["last_results"] = res
    return np.ascontiguousarray(
        np.concatenate([res.results[c]["out"] for c in range(NCORES)], axis=0)
    )
